# revision 1
# baseline (speedup 1.0000x reference)
"""Self-contained Trainium2 kernel for the SSD-scan actor network.

Data-parallel over batch B=8 across 8 NeuronCores (one sample per core, no
collectives). Per core:
  x  = relu(obs @ W_in + b_in)                  (T=512, D=2048)
  dt = softplus(x @ W_dt + dt_bias)             (T, H=16)
  Bm = x @ W_B, Cm = x @ W_C                    (T, H, N=64)
  y  = selective scan over T (Mamba2 SSD)       (T, D)
  z  = relu(y @ W_yo + b_yo)                    (T, U=256)
  out = z @ W_head + b_head                     (T, A=64)

The scan uses the chunked (segsum) SSD formulation: chunk length L=128,
4 chunks, 16 independent heads. Per head/chunk:
  E[j,i]  = exp(Pcum_i - Pcum_j + log dt_j), causally masked to j<=i
  Y^T     = x_chunk^T Gw + S_prev^T (C*u),  Gw = (B C^T)^T . E, u_i=exp(Pcum_i)
  S_new   = exp(Pcum_L-1) S_prev + sum_j exp(Pcum_L-1 - Pcum_j) dt_j B_j x_j^T
Big matmuls run in bf16 with fp32 PSUM accumulation; the Diff matrix
(Pcum_i - Pcum_j + logdt_j) is built exactly with K=2 fp32 matmuls.

Hardware notes (all discovered the hard way on this container's stack):
  - walrus here allows only ONE sync wait per instruction -> _split_multi_waits
  - matmul operands whose APs start at partition 64 crash the exec unit
    (NRT_EXEC_UNIT_UNRECOVERABLE), so every matmul operand is kept at base
    partition 0: B/C/Cw are repacked to 64-partition tensors via bf16
    staging + SBUF-to-SBUF DMA (DMA moves across partitions; DVE cannot).
  - Softplus shares no ACT function table with Exp/Ln -> ln(1+exp(x)).
"""

import sys
import types

import numpy as np
import ml_dtypes

T, BSZ, OBSD = 512, 8, 256
D, H, N, P = 2048, 16, 64, 128
U, A = 256, 64
L, NCH, KT = 128, 4, 16  # chunk length, #chunks, #d-tiles (D/128)
BF16 = ml_dtypes.bfloat16

_CACHE = {}
_EXECUTED = {}


def _patch_tile():
    """Split the TileContext final drain's waits across single-wait nops."""
    from concourse import tile, mybir
    from concourse.vector_clock import ScopedClock

    if getattr(tile.TileContext, "_drain_patched", False):
        return

    def _patched(self, tick_clock, wait_clock):
        nc = self.nc
        probe = nc.sync.nop()
        wait_clock.add_sem_waits(
            probe.ins, ScopedClock({None: tick_clock.global_clock})
        )
        si = probe.ins.sync_info
        if si is not None and len(si.on_wait) > 1:
            waits = list(si.on_wait)
            probe.ins.sync_info = mybir.SyncInfo(
                on_wait=[waits[0]], on_update=list(si.on_update)
            )
            for w in waits[1:]:
                nop = nc.sync.nop()
                nop.ins.sync_info = mybir.SyncInfo(on_wait=[w], on_update=[])
        nc.sync.drain()
        nc.all_engine_barrier(sem_only=True)
        assert self.sems is not None
        popped = nc._tile_sem_poison_stack.pop()
        assert popped is self._sem_poison
        nc.clear_and_free_semaphores(list(self.sems.allocated().values()))
        nc.all_engine_barrier(sem_only=True)

    tile.TileContext._drain_and_barrier = _patched
    tile.TileContext._drain_patched = True


def _split_multi_waits(nc):
    """This walrus build accepts at most one sync wait per instruction.
    Hoist extra waits onto single-wait NoOps inserted just before, on the
    same engine (the sequencer stalls there first — strictly conservative)."""
    from concourse import mybir

    n = 0
    for f in nc.m.functions:
        for bb in f.blocks:
            insts = list(bb.instructions)
            changed = False
            new = []
            for inst in insts:
                try:
                    si = inst.sync_info
                except Exception:
                    si = None
                if si is not None and len(si.on_wait) > 1:
                    waits = list(si.on_wait)
                    for w in waits[:-1]:
                        nop = mybir.InstNoOp(
                            name=f"wsplit-{n}", ins=[], outs=[], engine=inst.engine
                        )
                        n += 1
                        nop.sync_info = mybir.SyncInfo(on_wait=[w], on_update=[])
                        nc.register_instruction(nop, overwrite=True)
                        new.append(nop)
                    inst.sync_info = mybir.SyncInfo(
                        on_wait=[waits[-1]], on_update=list(si.on_update)
                    )
                    changed = True
                new.append(inst)
            if changed:
                bb.instructions = new


def _inject_axon_hooks():
    """Make trace=True work (and a BASS_TRACE env var safe) in this container."""
    if "antenv.axon_hooks" not in sys.modules:
        try:
            from trn_agent_boot.trn_boot import _ntff_profile_via_ctypes

            hook = _ntff_profile_via_ctypes("/opt/axon/libaxon_pjrt.so")
        except Exception:
            hook = None
        mod = types.ModuleType("antenv.axon_hooks")
        mod.get_axon_ntff_profile_hook = lambda: hook
        mod.set_axon_ntff_profile_hook = lambda h: None
        sys.modules["antenv.axon_hooks"] = mod
    from concourse import bass_utils

    bass_utils.upload_artifacts = lambda d: d



def _build(with_b_in, with_b_yo, with_b_head):
    import concourse.bass as bass
    import concourse.mybir as mybir
    from concourse.tile import TileContext
    from concourse.masks import make_identity

    dt = mybir.dt
    AF = mybir.ActivationFunctionType
    OP = mybir.AluOpType

    nc = bass.Bass()
    obsT_e = nc.declare_dram_parameter("obsT", [OBSD, T], dt.bfloat16, isOutput=False)
    w_in_e = nc.declare_dram_parameter("w_in", [OBSD, D], dt.bfloat16, isOutput=False)
    w_dt_e = nc.declare_dram_parameter("w_dt", [D, H], dt.bfloat16, isOutput=False)
    w_b_e = nc.declare_dram_parameter("w_b", [D, H * N], dt.bfloat16, isOutput=False)
    w_c_e = nc.declare_dram_parameter("w_c", [D, H * N], dt.bfloat16, isOutput=False)
    w_yo_e = nc.declare_dram_parameter("w_yo", [D, U], dt.bfloat16, isOutput=False)
    w_hd_e = nc.declare_dram_parameter("w_hd", [U, A], dt.bfloat16, isOutput=False)
    neg_a_e = nc.declare_dram_parameter("neg_a", [H, 1], dt.float32, isOutput=False)
    dtb_e = nc.declare_dram_parameter("dtb", [H, 1], dt.float32, isOutput=False)
    bin_e = binr_e = byo_e = bhd_e = None
    if with_b_in:
        bin_e = nc.declare_dram_parameter("b_in", [P, KT], dt.float32, isOutput=False)
        binr_e = nc.declare_dram_parameter("b_in_r", [1, D], dt.bfloat16, isOutput=False)
    if with_b_yo:
        byo_e = nc.declare_dram_parameter("b_yo", [P, 2], dt.float32, isOutput=False)
    if with_b_head:
        bhd_e = nc.declare_dram_parameter("b_hd", [1, A], dt.bfloat16, isOutput=False)
    out_e = nc.declare_dram_parameter("out", [T, A], dt.float32, isOutput=True)

    _patch_tile()
    with TileContext(nc) as tc:
        with (
            tc.tile_pool(name="cst", bufs=1) as cst,
            tc.tile_pool(name="wrk", bufs=2) as wrk,
            tc.tile_pool(name="ps_proj", bufs=2, space="PSUM") as ps_proj,
            tc.tile_pool(name="ps_diff", bufs=2, space="PSUM") as ps_diff,
            tc.tile_pool(name="ps_gy", bufs=2, space="PSUM") as ps_gy,
            tc.tile_pool(name="ps_sd", bufs=1, space="PSUM") as ps_sd,
            tc.tile_pool(name="ps_tiny", bufs=1, space="PSUM") as ps_tiny,
        ):
            # ---------------- weights + constants -----------------
            obsT = cst.tile([P, 2 * T], dt.bfloat16, tag="obsT")
            for k in range(2):
                nc.sync.dma_start(
                    out=obsT[:, k * T : (k + 1) * T], in_=obsT_e[k * P : (k + 1) * P, :]
                )
            w_in = cst.tile([P, 2 * D], dt.bfloat16, tag="w_in")
            for k in range(2):
                nc.sync.dma_start(
                    out=w_in[:, k * D : (k + 1) * D], in_=w_in_e[k * P : (k + 1) * P, :]
                )
            w_dt = cst.tile([P, KT * H], dt.bfloat16, tag="w_dt")
            for k in range(KT):
                nc.sync.dma_start(
                    out=w_dt[:, k * H : (k + 1) * H], in_=w_dt_e[k * P : (k + 1) * P, :]
                )
            w_yo = cst.tile([P, KT * U], dt.bfloat16, tag="w_yo")
            for k in range(KT):
                nc.sync.dma_start(
                    out=w_yo[:, k * U : (k + 1) * U], in_=w_yo_e[k * P : (k + 1) * P, :]
                )
            w_hd = cst.tile([P, 2 * A], dt.bfloat16, tag="w_hd")
            for k in range(2):
                nc.sync.dma_start(
                    out=w_hd[:, k * A : (k + 1) * A], in_=w_hd_e[k * P : (k + 1) * P, :]
                )
            neg_a = cst.tile([H, 1], dt.float32, tag="neg_a")
            nc.sync.dma_start(out=neg_a[:], in_=neg_a_e[:])
            dtb = cst.tile([H, 1], dt.float32, tag="dtb")
            nc.sync.dma_start(out=dtb[:], in_=dtb_e[:])
            if with_b_in:
                b_in = cst.tile([P, KT], dt.float32, tag="b_in")
                nc.sync.dma_start(out=b_in[:], in_=bin_e[:])
                b_in_r = cst.tile([1, D], dt.bfloat16, tag="b_in_r")
                nc.sync.dma_start(out=b_in_r[:], in_=binr_e[:])
            if with_b_yo:
                b_yo = cst.tile([P, 2], dt.float32, tag="b_yo")
                nc.sync.dma_start(out=b_yo[:], in_=byo_e[:])
            if with_b_head:
                b_hd = cst.tile([1, A], dt.bfloat16, tag="b_hd")
                nc.sync.dma_start(out=b_hd[:], in_=bhd_e[:])

            ident_f = cst.tile([P, P], dt.float32, tag="ident_f")
            make_identity(nc, ident_f[:])
            ident_b = cst.tile([N, N], dt.bfloat16, tag="ident_b")
            make_identity(nc, ident_b[:])
            # causal ones: UT[j,i] = 1 where j<=i (cumsum matmul)
            ut_ones = cst.tile([L, L], dt.float32, tag="ut_ones")
            nc.gpsimd.memset(ut_ones[:], 1.0)
            nc.gpsimd.affine_select(
                out=ut_ones[:],
                in_=ut_ones[:],
                compare_op=OP.is_ge,
                fill=0.0,
                base=0,
                pattern=[[1, L]],
                channel_multiplier=-1,
            )
            ones_row = cst.tile([1, T], dt.float32, tag="ones_row")
            nc.gpsimd.memset(ones_row[:], 1.0)
            ones_bf = cst.tile([1, T], dt.bfloat16, tag="ones_bf")
            nc.gpsimd.memset(ones_bf[:], 1.0)

            # ---------------- activations / state ------------------
            xT = cst.tile([P, KT * T], dt.bfloat16, tag="xT")  # (d, t)
            x = cst.tile([P, NCH * D], dt.bfloat16, tag="x")  # (t, d) per t-tile
            # B/C/Cw repacked: 64 partitions, head h at columns [h*T,(h+1)*T)
            bm = cst.tile([N, H * T], dt.bfloat16, tag="bm")
            cm = cst.tile([N, H * T], dt.bfloat16, tag="cm")
            cw = cst.tile([N, H * T], dt.bfloat16, tag="cw")
            y = cst.tile([P, KT * T], dt.bfloat16, tag="y")  # (d, t) per d-tile
            zT = cst.tile([P, 2 * T], dt.bfloat16, tag="zT")  # (u, t)
            logit = cst.tile([P, NCH * A], dt.float32, tag="logit")
            s_st = [
                cst.tile([N, P], dt.bfloat16, tag=f"s{h}", name=f"s{h}")
                for h in range(H)
            ]
            for h in range(H):
                nc.gpsimd.memset(s_st[h][:], 0.0)

            dtT = cst.tile([H, T], dt.float32, tag="dtT")
            logdtT = cst.tile([H, T], dt.float32, tag="logdtT")
            pcumT = cst.tile([H, T], dt.float32, tag="pcumT")  # Pcum rows (h,t)
            npdT = cst.tile([H, T], dt.float32, tag="npdT")  # logdt - Pcum (h,t)
            # two-float (hi/lo bf16) splits for the K=4 bf16 Diff matmul
            pcumH = cst.tile([H, T], dt.bfloat16, tag="pcumH")
            pcumL = cst.tile([H, T], dt.bfloat16, tag="pcumL")
            npdH = cst.tile([H, T], dt.bfloat16, tag="npdH")
            npdL = cst.tile([H, T], dt.bfloat16, tag="npdL")
            plrow = cst.tile([1, NCH * H], dt.float32, tag="plrow")
            u_all = cst.tile([H, T], dt.bfloat16, tag="u_all")  # exp(Pcum)
            cols = cst.tile([P, NCH * 2 * H], dt.float32, tag="cols")  # [PcumCol|dtCol]
            e2c = cst.tile([P, NCH * H], dt.float32, tag="e2c")  # exp(Plast-Pcum)
            dtotc = cst.tile([P, NCH * H], dt.float32, tag="dtotc")
            ulast = cst.tile([1, NCH * H], dt.float32, tag="ulast")

            # ---------------- x^T = relu(W_in^T obs^T) (d,t) --------
            for kt in range(KT):
                ps = ps_proj.tile([P, T], dt.float32, tag="proj")
                for ko in range(2):
                    nc.tensor.matmul(
                        ps[:],
                        w_in[:, ko * D + kt * P : ko * D + (kt + 1) * P],
                        obsT[:, ko * T : (ko + 1) * T],
                        start=(ko == 0),
                        stop=(ko == 1),
                    )
                if with_b_in:
                    nc.scalar.activation(
                        xT[:, kt * T : (kt + 1) * T], ps[:], AF.Relu,
                        bias=b_in[:, kt : kt + 1],
                    )
                else:
                    nc.scalar.activation(xT[:, kt * T : (kt + 1) * T], ps[:], AF.Relu)

            # ---------------- x = relu(obs W_in) (t,d) --------------
            for tt in range(NCH):
                for ds in range(4):
                    ps = ps_proj.tile([P, T], dt.float32, tag="proj")
                    nmm = 3 if with_b_in else 2
                    for ko in range(2):
                        nc.tensor.matmul(
                            ps[:],
                            obsT[:, ko * T + tt * P : ko * T + (tt + 1) * P],
                            w_in[:, ko * D + ds * 512 : ko * D + (ds + 1) * 512],
                            start=(ko == 0),
                            stop=(ko == nmm - 1),
                        )
                    if with_b_in:
                        nc.tensor.matmul(
                            ps[:],
                            ones_bf[0:1, 0:P],
                            b_in_r[0:1, ds * 512 : (ds + 1) * 512],
                            start=False,
                            stop=True,
                        )
                    nc.vector.tensor_scalar_max(
                        x[:, tt * D + ds * 512 : tt * D + (ds + 1) * 512], ps[:], 0.0
                    )

            # ---------------- dt chain ------------------------------
            psd = ps_proj.tile([H, T], dt.float32, tag="proj", name="psd")
            for kt in range(KT):
                nc.tensor.matmul(
                    psd[:],
                    w_dt[:, kt * H : (kt + 1) * H],
                    xT[:, kt * T : (kt + 1) * T],
                    start=(kt == 0),
                    stop=(kt == KT - 1),
                )
            # softplus via ln(1+exp(.)) — Softplus shares no ACT table with
            # Exp/Ln on this compiler; exp/ln/relu/copy live in one table.
            ez = wrk.tile([H, T], dt.float32, tag="ez")
            nc.scalar.activation(ez[:], psd[:], AF.Exp, bias=dtb[:])
            nc.vector.tensor_scalar_add(ez[:], ez[:], 1.0)
            nc.scalar.activation(dtT[:], ez[:], AF.Ln)
            nc.scalar.activation(logdtT[:], dtT[:], AF.Ln)

            for c in range(NCH):
                cb = slice(c * L, (c + 1) * L)
                ldec = wrk.tile([H, L], dt.float32, tag="ldec")
                nc.vector.tensor_scalar_mul(ldec[:], dtT[:, cb], neg_a[:])
                pt = ps_tiny.tile([P, 2 * H], dt.float32, tag="tiny")
                nc.tensor.transpose(pt[:, 0:H], ldec[:], ident_f[0:H, 0:H])
                ldec_c = wrk.tile([P, H], dt.float32, tag="ldec_c")
                nc.vector.tensor_copy(ldec_c[:], pt[:, 0:H])
                pp = ps_tiny.tile([H, L], dt.float32, tag="tiny")
                nc.tensor.matmul(pp[:], ldec_c[:], ut_ones[:], start=True, stop=True)
                nc.vector.tensor_copy(pcumT[:, cb], pp[:])
                nc.vector.tensor_sub(npdT[:, cb], logdtT[:, cb], pcumT[:, cb])
                nc.vector.tensor_copy(pcumH[:, cb], pcumT[:, cb])
                nc.vector.tensor_sub(pcumL[:, cb], pcumT[:, cb], pcumH[:, cb])
                nc.vector.tensor_copy(npdH[:, cb], npdT[:, cb])
                nc.vector.tensor_sub(npdL[:, cb], npdT[:, cb], npdH[:, cb])
                nc.scalar.activation(u_all[:, cb], pcumT[:, cb], AF.Exp)
                pt2 = ps_tiny.tile([P, 2 * H], dt.float32, tag="tiny")
                nc.tensor.transpose(pt2[:, 0:H], pcumT[:, cb], ident_f[0:H, 0:H])
                nc.tensor.transpose(pt2[:, H : 2 * H], dtT[:, cb], ident_f[0:H, 0:H])
                co = c * 2 * H
                nc.vector.tensor_copy(cols[:, co : co + 2 * H], pt2[:])
                # PcumLast per head at base partition 0 (row 127 of PcumCol)
                nc.sync.dma_start(
                    out=plrow[:, c * H : (c + 1) * H],
                    in_=cols[L - 1 : L, co : co + H],
                )
                plast = plrow[:, c * H : (c + 1) * H]
                nc.scalar.activation(ulast[:, c * H : (c + 1) * H], plast, AF.Exp)
                pdt = ps_tiny.tile([P, H], dt.float32, tag="tiny")
                nc.tensor.matmul(
                    pdt[:],
                    ones_row[0:1, 0:P],
                    ulast[:, c * H : (c + 1) * H],
                    start=True,
                    stop=True,
                )
                nc.vector.tensor_copy(dtotc[:, c * H : (c + 1) * H], pdt[:])
                ppl = ps_tiny.tile([P, H], dt.float32, tag="tiny")
                nc.tensor.matmul(
                    ppl[:], ones_row[0:1, 0:P], plast, start=True, stop=True
                )
                e2a = wrk.tile([P, H], dt.float32, tag="e2a")
                nc.vector.tensor_sub(e2a[:], ppl[:], cols[:, co : co + H])
                nc.scalar.activation(e2c[:, c * H : (c + 1) * H], e2a[:], AF.Exp)

            # ---------------- B / C projections ---------------------
            # psum (128, T) holds heads (2mt, 2mt+1); evacuate to bf16 staging
            # then DMA each 64-partition half into the base-0 packed tensors.
            wbv = w_b_e.rearrange("(kt p) m -> p kt m", p=P)
            wcv = w_c_e.rearrange("(kt p) m -> p kt m", p=P)
            for mt in range(8):
                he, ho = 2 * mt, 2 * mt + 1
                wbuf = wrk.tile([P, KT * P], dt.bfloat16, tag="wbs")
                nc.sync.dma_start(
                    out=wbuf[:].rearrange("p (kt m) -> p kt m", kt=KT),
                    in_=wbv[:, :, mt * P : (mt + 1) * P],
                )
                psb = ps_proj.tile([P, T], dt.float32, tag="proj")
                for kt in range(KT):
                    nc.tensor.matmul(
                        psb[:],
                        wbuf[:, kt * P : (kt + 1) * P],
                        xT[:, kt * T : (kt + 1) * T],
                        start=(kt == 0),
                        stop=(kt == KT - 1),
                    )
                btmp = wrk.tile([P, T], dt.bfloat16, tag="btmp", bufs=3)
                nc.vector.tensor_copy(btmp[:], psb[:])
                nc.sync.dma_start(out=bm[:, he * T : (he + 1) * T], in_=btmp[0:N, :])
                nc.sync.dma_start(out=bm[:, ho * T : (ho + 1) * T], in_=btmp[N:P, :])

                wcuf = wrk.tile([P, KT * P], dt.bfloat16, tag="wcs")
                nc.sync.dma_start(
                    out=wcuf[:].rearrange("p (kt m) -> p kt m", kt=KT),
                    in_=wcv[:, :, mt * P : (mt + 1) * P],
                )
                psc = ps_proj.tile([P, T], dt.float32, tag="proj")
                for kt in range(KT):
                    nc.tensor.matmul(
                        psc[:],
                        wcuf[:, kt * P : (kt + 1) * P],
                        xT[:, kt * T : (kt + 1) * T],
                        start=(kt == 0),
                        stop=(kt == KT - 1),
                    )
                ctmp = wrk.tile([P, T], dt.bfloat16, tag="ctmp", bufs=3)
                nc.vector.tensor_copy(ctmp[:], psc[:])
                nc.sync.dma_start(out=cm[:, he * T : (he + 1) * T], in_=ctmp[0:N, :])
                nc.sync.dma_start(out=cm[:, ho * T : (ho + 1) * T], in_=ctmp[N:P, :])

                # u-scaled C: broadcast u rows across partitions via K=1 matmuls
                urow = wrk.tile([1, 2 * T], dt.bfloat16, tag="urow")
                nc.sync.dma_start(out=urow[:, 0:T], in_=u_all[he : he + 1, :])
                nc.sync.dma_start(out=urow[:, T : 2 * T], in_=u_all[ho : ho + 1, :])
                ubp = ps_proj.tile([P, T], dt.float32, tag="proj", name="ubp")
                nc.tensor.matmul(
                    ubp[0:N, :], ones_bf[0:1, 0:N], urow[:, 0:T],
                    start=True, stop=True,
                )
                nc.tensor.matmul(
                    ubp[N:P, :], ones_bf[0:1, 0:N], urow[:, T : 2 * T],
                    start=True, stop=True,
                )
                ubc = wrk.tile([P, T], dt.bfloat16, tag="ubc")
                nc.scalar.activation(ubc[:], ubp[:], AF.Copy)
                wtmp = wrk.tile([P, T], dt.bfloat16, tag="wtmp", bufs=3)
                nc.vector.tensor_mul(wtmp[:], psc[:], ubc[:])
                nc.sync.dma_start(out=cw[:, he * T : (he + 1) * T], in_=wtmp[0:N, :])
                nc.sync.dma_start(out=cw[:, ho * T : (ho + 1) * T], in_=wtmp[N:P, :])

            # ---------------- scan ----------------------------------
            yv = y[:].rearrange("p (h t) -> p h t", h=KT)  # (128, 16, 512)
            # chunk-major: consecutive groups touch different heads, so the
            # serial per-head state chain is 4 groups apart and the scan
            # pipelines without waiting on the S-update chain.
            for c in range(NCH):
                for hg in range(4):
                    cb = slice(c * L, (c + 1) * L)
                    # 2-row Diff-matmul packs for this head group / chunk:
                    # lh2g row0 = ones, row1 = (logdt - Pcum); rp2g row0 = Pcum,
                    # row1 = ones; 4 heads side by side along free.
                    # K=4 bf16 Diff packs: lh rows [1, npdH, 1, npdL],
                    # rp rows [pcumH, 1, pcumL, 1]; 4 heads along free.
                    lh2g = wrk.tile([4, 4 * L], dt.bfloat16, tag="lh2g")
                    rp2g = wrk.tile([4, 4 * L], dt.bfloat16, tag="rp2g")
                    nc.gpsimd.memset(lh2g[:], 1.0)
                    nc.gpsimd.memset(rp2g[:], 1.0)
                    hgs = slice(hg * 4, hg * 4 + 4)
                    nc.sync.dma_start(
                        out=rp2g[0:1, :].rearrange("p (h t) -> p h t", h=4),
                        in_=pcumH[hgs, cb],
                    )
                    nc.sync.dma_start(
                        out=rp2g[2:3, :].rearrange("p (h t) -> p h t", h=4),
                        in_=pcumL[hgs, cb],
                    )
                    nc.sync.dma_start(
                        out=lh2g[1:2, :].rearrange("p (h t) -> p h t", h=4),
                        in_=npdH[hgs, cb],
                    )
                    nc.sync.dma_start(
                        out=lh2g[3:4, :].rearrange("p (h t) -> p h t", h=4),
                        in_=npdL[hgs, cb],
                    )
                    dbank = ps_diff.tile([P, 4 * L], dt.float32, tag="diff")
                    gbank = ps_gy.tile([P, 4 * L], dt.float32, tag="gy")
                    for hi in range(4):
                        h = hg * 4 + hi
                        hb = slice(h * T + c * L, h * T + (c + 1) * L)
                        nc.tensor.matmul(
                            dbank[:, hi * L : (hi + 1) * L],
                            lh2g[:, hi * L : (hi + 1) * L],
                            rp2g[:, hi * L : (hi + 1) * L],
                            start=True,
                            stop=True,
                        )
                        nc.tensor.matmul(
                            gbank[:, hi * L : (hi + 1) * L],
                            bm[:, hb],
                            cm[:, hb],
                            start=True,
                            stop=True,
                        )
                    e_sb = wrk.tile([P, 4 * L], dt.float32, tag="e_sb", bufs=3)
                    nc.scalar.activation(e_sb[:], dbank[:], AF.Exp)
                    # causal mask: keep i>=j else 0 (kills the exp-overflow infs)
                    nc.gpsimd.affine_select(
                        out=e_sb[:],
                        in_=e_sb[:],
                        compare_op=OP.is_ge,
                        fill=0.0,
                        base=0,
                        pattern=[[0, 4], [1, L]],
                        channel_multiplier=-1,
                    )
                    gw = wrk.tile([P, 4 * L], dt.bfloat16, tag="gw", bufs=3)
                    nc.vector.tensor_mul(gw[:], gbank[:], e_sb[:])

                    ybank = ps_gy.tile([P, 4 * L], dt.float32, tag="gy")
                    btr = ps_tiny.tile([P, 4 * N], dt.bfloat16, tag="tiny")
                    sdb = ps_sd.tile([N, 4 * P], dt.float32, tag="sd")
                    bd = wrk.tile([P, 4 * N], dt.bfloat16, tag="bd")
                    for hi in range(4):
                        h = hg * 4 + hi
                        hb = slice(h * T + c * L, h * T + (c + 1) * L)
                        xc = x[:, c * D + h * P : c * D + (h + 1) * P]
                        nc.tensor.matmul(
                            ybank[:, hi * L : (hi + 1) * L],
                            xc,
                            gw[:, hi * L : (hi + 1) * L],
                            start=True,
                            stop=False,
                        )
                        nc.tensor.matmul(
                            ybank[:, hi * L : (hi + 1) * L],
                            s_st[h][:],
                            cw[:, hb],
                            start=False,
                            stop=True,
                        )
                        nc.tensor.transpose(
                            btr[:, hi * N : (hi + 1) * N],
                            bm[:, hb],
                            ident_b[:],
                        )
                        nc.vector.tensor_scalar(
                            bd[:, hi * N : (hi + 1) * N],
                            btr[:, hi * N : (hi + 1) * N],
                            e2c[:, c * H + h : c * H + h + 1],
                            cols[:, c * 2 * H + H + h : c * 2 * H + H + h + 1],
                            op0=OP.mult,
                            op1=OP.mult,
                        )
                        sds = sdb[:, hi * P : (hi + 1) * P]
                        nc.tensor.matmul(
                            sds, bd[:, hi * N : (hi + 1) * N], xc,
                            start=True, stop=True,
                        )
                        nc.vector.scalar_tensor_tensor(
                            s_st[h][:],
                            s_st[h][:],
                            dtotc[0:N, c * H + h : c * H + h + 1],
                            sds,
                            op0=OP.mult,
                            op1=OP.add,
                        )
                    # Y evac: psum (p, 4*L) -> y (d,t) blocks [h, c*L:(c+1)*L]
                    nc.scalar.activation(
                        yv[:, hg * 4 : hg * 4 + 4, cb],
                        ybank[:].rearrange("p (h t) -> p h t", h=4),
                        AF.Copy,
                    )

            # ---------------- z = relu(y W_yo) (u,t) ----------------
            for ut in range(2):
                ps = ps_proj.tile([P, T], dt.float32, tag="proj")
                for kt in range(KT):
                    nc.tensor.matmul(
                        ps[:],
                        w_yo[:, kt * U + ut * P : kt * U + (ut + 1) * P],
                        y[:, kt * T : (kt + 1) * T],
                        start=(kt == 0),
                        stop=(kt == KT - 1),
                    )
                if with_b_yo:
                    nc.scalar.activation(
                        zT[:, ut * T : (ut + 1) * T], ps[:], AF.Relu,
                        bias=b_yo[:, ut : ut + 1],
                    )
                else:
                    nc.scalar.activation(zT[:, ut * T : (ut + 1) * T], ps[:], AF.Relu)

            # ---------------- logits --------------------------------
            for tt in range(NCH):
                ps = ps_proj.tile([P, A], dt.float32, tag="proj")
                nmm = 3 if with_b_head else 2
                for ut in range(2):
                    nc.tensor.matmul(
                        ps[:],
                        zT[:, ut * T + tt * P : ut * T + (tt + 1) * P],
                        w_hd[:, ut * A : (ut + 1) * A],
                        start=(ut == 0),
                        stop=(ut == nmm - 1),
                    )
                if with_b_head:
                    nc.tensor.matmul(
                        ps[:],
                        ones_bf[0:1, tt * P : (tt + 1) * P],
                        b_hd[:],
                        start=False,
                        stop=True,
                    )
                nc.scalar.activation(logit[:, tt * A : (tt + 1) * A], ps[:], AF.Copy)
                nc.sync.dma_start(
                    out=out_e[tt * P : (tt + 1) * P, :],
                    in_=logit[:, tt * A : (tt + 1) * A],
                )

    _split_multi_waits(nc)
    return nc


def kernel(obs, W_in, b_in, A_log, dt_bias, W_dt, W_B, W_C, W_yo, b_yo, W_head, b_head):
    _inject_axon_hooks()
    _patch_tile()
    from concourse.bass_utils import run_bass_kernel_spmd

    obs = np.asarray(obs, dtype=np.float32)
    flags = (
        bool(np.any(np.asarray(b_in) != 0)),
        bool(np.any(np.asarray(b_yo) != 0)),
        bool(np.any(np.asarray(b_head) != 0)),
    )
    # First call: build once (the verified path). Repeat calls in one
    # process rebuild a fresh graph — re-executing a previously-run nc with
    # new inputs has crashed the exec unit (NRT status 101) in testing.
    if flags not in _CACHE:
        _CACHE[flags] = _build(*flags)
    elif _EXECUTED.get(flags):
        _CACHE[flags] = _build(*flags)
    nc = _CACHE[flags]
    _EXECUTED[flags] = True

    obsT = obs.reshape(T, BSZ, OBSD).transpose(1, 2, 0)  # (B, 256, T)
    base = {
        "w_in": np.ascontiguousarray(W_in).astype(BF16),
        "w_dt": np.ascontiguousarray(W_dt).astype(BF16),
        "w_b": np.ascontiguousarray(W_B).astype(BF16),
        "w_c": np.ascontiguousarray(W_C).astype(BF16),
        "w_yo": np.ascontiguousarray(W_yo).astype(BF16),
        "w_hd": np.ascontiguousarray(W_head).astype(BF16),
        "neg_a": (-np.exp(np.asarray(A_log, np.float64)))
        .astype(np.float32)
        .reshape(H, 1),
        "dtb": np.asarray(dt_bias, np.float32).reshape(H, 1),
    }
    if flags[0]:
        base["b_in"] = np.ascontiguousarray(
            np.asarray(b_in, np.float32).reshape(KT, P).T
        )
        base["b_in_r"] = np.asarray(b_in).astype(BF16).reshape(1, D)
    if flags[1]:
        base["b_yo"] = np.ascontiguousarray(
            np.asarray(b_yo, np.float32).reshape(2, P).T
        )
    if flags[2]:
        base["b_hd"] = np.asarray(b_head).astype(BF16).reshape(1, A)
    in_maps = [
        dict(base, obsT=np.ascontiguousarray(obsT[c]).astype(BF16)) for c in range(BSZ)
    ]
    global _last_in_maps
    _last_in_maps = in_maps
    res = run_bass_kernel_spmd(nc, in_maps, core_ids=list(range(BSZ)))
    out = np.stack([res.results[c]["out"] for c in range(BSZ)], axis=1)
    return out.astype(np.float32)



# revision 5
# speedup vs baseline: 1.0236x; 1.0236x over previous
"""Self-contained Trainium2 kernel for the SSD-scan actor network.

Data-parallel over batch B=8 across 8 NeuronCores (one sample per core, no
collectives). Per core:
  x  = relu(obs @ W_in + b_in)                  (T=512, D=2048)
  dt = softplus(x @ W_dt + dt_bias)             (T, H=16)
  Bm = x @ W_B, Cm = x @ W_C                    (T, H, N=64)
  y  = selective scan over T (Mamba2 SSD)       (T, D)
  z  = relu(y @ W_yo + b_yo)                    (T, U=256)
  out = z @ W_head + b_head                     (T, A=64)

The scan uses the chunked (segsum) SSD formulation: chunk length L=128,
4 chunks, 16 independent heads. Per head/chunk:
  E[j,i]  = exp(Pcum_i - Pcum_j + log dt_j), causally masked to j<=i
  Y^T     = x_chunk^T Gw + S_prev^T (C*u),  Gw = (B C^T)^T . E, u_i=exp(Pcum_i)
  S_new   = exp(Pcum_L-1) S_prev + sum_j exp(Pcum_L-1 - Pcum_j) dt_j B_j x_j^T
Big matmuls run in bf16 with fp32 PSUM accumulation; the Diff matrix
(Pcum_i - Pcum_j + logdt_j) is built exactly in bf16 hi/lo splits via ONE
K=16 block-diagonal matmul per 4-head group (operands bulk-staged once).

v2 perf restructuring vs the first working version:
  - scan heads regrouped as h = 4*hi + hg so the Diff operands for all 16
    (chunk, group) tiles stage with 16 medium DMAs instead of 64 tiny
    just-in-time DMAs + 32 memsets on the critical path.
  - x (t-major) derived from xT by PE transposes (saves 8k matmul cols).
  - scan emission is software-pipelined: group g's Diff+G matmuls are
    emitted before group g-1's Y/S-update block, so the PE never sits in
    the exp->mask->mul latency chain (this idling used to re-throttle the
    PE clock to 1.2 GHz via HAM for ~half the kernel).
  - W_B/W_C streamed with 3-deep prefetch, loads split across the two
    HWDGE issue engines (sync + scalar).
  - z-projection accumulates per-head during the last scan chunk; SBUF
    reuse: y aliases xT, zT aliases obsT, scan rings carve dead w_in.

Hardware notes (all discovered the hard way on this container's stack):
  - walrus here allows only ONE sync wait per instruction -> _split_multi_waits
  - matmul operands whose APs start at partition 64 crash the exec unit
    (NRT_EXEC_UNIT_UNRECOVERABLE), so every matmul operand is kept at base
    partition 0: B/C/Cw are repacked to 64-partition tensors via bf16
    staging + SBUF-to-SBUF DMA (DMA moves across partitions; DVE cannot).
  - Softplus shares no ACT function table with Exp/Ln -> ln(1+exp(x)).
"""

import sys
import types

import numpy as np
import ml_dtypes

T, BSZ, OBSD = 512, 8, 256
D, H, N, P = 2048, 16, 64, 128
U, A = 256, 64
L, NCH, KT = 128, 4, 16  # chunk length, #chunks, #d-tiles (D/128)
HN = H * N
BF16 = ml_dtypes.bfloat16

_CACHE = {}
_EXECUTED = {}


def _patch_tile():
    """Split the TileContext final drain's waits across single-wait nops."""
    from concourse import tile, mybir
    from concourse.vector_clock import ScopedClock

    if getattr(tile.TileContext, "_drain_patched", False):
        return

    def _patched(self, tick_clock, wait_clock):
        nc = self.nc
        probe = nc.sync.nop()
        wait_clock.add_sem_waits(
            probe.ins, ScopedClock({None: tick_clock.global_clock})
        )
        si = probe.ins.sync_info
        if si is not None and len(si.on_wait) > 1:
            waits = list(si.on_wait)
            probe.ins.sync_info = mybir.SyncInfo(
                on_wait=[waits[0]], on_update=list(si.on_update)
            )
            for w in waits[1:]:
                nop = nc.sync.nop()
                nop.ins.sync_info = mybir.SyncInfo(on_wait=[w], on_update=[])
        nc.sync.drain()
        nc.all_engine_barrier(sem_only=True)
        assert self.sems is not None
        popped = nc._tile_sem_poison_stack.pop()
        assert popped is self._sem_poison
        nc.clear_and_free_semaphores(list(self.sems.allocated().values()))
        nc.all_engine_barrier(sem_only=True)

    tile.TileContext._drain_and_barrier = _patched
    tile.TileContext._drain_patched = True


def _split_multi_waits(nc):
    """This walrus build accepts at most one sync wait per instruction.
    Hoist extra waits onto single-wait NoOps inserted just before, on the
    same engine (the sequencer stalls there first — strictly conservative)."""
    from concourse import mybir

    n = 0
    for f in nc.m.functions:
        for bb in f.blocks:
            insts = list(bb.instructions)
            changed = False
            new = []
            for inst in insts:
                try:
                    si = inst.sync_info
                except Exception:
                    si = None
                if si is not None and len(si.on_wait) > 1:
                    waits = list(si.on_wait)
                    for w in waits[:-1]:
                        nop = mybir.InstNoOp(
                            name=f"wsplit-{n}", ins=[], outs=[], engine=inst.engine
                        )
                        n += 1
                        nop.sync_info = mybir.SyncInfo(on_wait=[w], on_update=[])
                        nc.register_instruction(nop, overwrite=True)
                        new.append(nop)
                    inst.sync_info = mybir.SyncInfo(
                        on_wait=[waits[-1]], on_update=list(si.on_update)
                    )
                    changed = True
                new.append(inst)
            if changed:
                bb.instructions = new


def _inject_axon_hooks():
    """Make trace=True work (and a BASS_TRACE env var safe) in this container."""
    if "antenv.axon_hooks" not in sys.modules:
        try:
            from trn_agent_boot.trn_boot import _ntff_profile_via_ctypes

            hook = _ntff_profile_via_ctypes("/opt/axon/libaxon_pjrt.so")
        except Exception:
            hook = None
        mod = types.ModuleType("antenv.axon_hooks")
        mod.get_axon_ntff_profile_hook = lambda: hook
        mod.set_axon_ntff_profile_hook = lambda h: None
        sys.modules["antenv.axon_hooks"] = mod
    from concourse import bass_utils

    bass_utils.upload_artifacts = lambda d: d


def _build(with_b_in, with_b_yo, with_b_head):
    import concourse.bass as bass
    import concourse.mybir as mybir
    from concourse.tile import TileContext
    from concourse.masks import make_identity

    dt = mybir.dt
    AF = mybir.ActivationFunctionType
    OP = mybir.AluOpType

    nc = bass.Bass()
    obsT_e = nc.declare_dram_parameter("obsT", [OBSD, T], dt.bfloat16, isOutput=False)
    w_in_e = nc.declare_dram_parameter("w_in", [OBSD, D], dt.bfloat16, isOutput=False)
    w_dt_e = nc.declare_dram_parameter("w_dt", [D, H], dt.bfloat16, isOutput=False)
    w_b_e = nc.declare_dram_parameter("w_b", [D, HN], dt.bfloat16, isOutput=False)
    w_c_e = nc.declare_dram_parameter("w_c", [D, HN], dt.bfloat16, isOutput=False)
    w_yo_e = nc.declare_dram_parameter("w_yo", [D, U], dt.bfloat16, isOutput=False)
    w_hd_e = nc.declare_dram_parameter("w_hd", [U, A], dt.bfloat16, isOutput=False)
    neg_a_e = nc.declare_dram_parameter("neg_a", [H, 1], dt.float32, isOutput=False)
    dtb_e = nc.declare_dram_parameter("dtb", [H, 1], dt.float32, isOutput=False)
    bin_e = byo_e = bhd_e = None
    if with_b_in:
        bin_e = nc.declare_dram_parameter("b_in", [P, KT], dt.float32, isOutput=False)
    if with_b_yo:
        byo_e = nc.declare_dram_parameter("b_yo", [P, 2], dt.float32, isOutput=False)
    if with_b_head:
        bhd_e = nc.declare_dram_parameter("b_hd", [1, A], dt.bfloat16, isOutput=False)
    out_e = nc.declare_dram_parameter("out", [T, A], dt.float32, isOutput=True)

    _patch_tile()
    with TileContext(nc) as tc:
        with (
            tc.tile_pool(name="cst", bufs=1) as cst,
            tc.tile_pool(name="wrk", bufs=2) as wrk,
            tc.tile_pool(name="ps_proj", bufs=2, space="PSUM") as ps_proj,
            tc.tile_pool(name="ps_diff", bufs=2, space="PSUM") as ps_diff,
            tc.tile_pool(name="ps_gy", bufs=2, space="PSUM") as ps_gy,
            tc.tile_pool(name="ps_sd", bufs=1, space="PSUM") as ps_sd,
            tc.tile_pool(name="ps_tiny", bufs=1, space="PSUM") as ps_tiny,
        ):
            # ---------------- weights + constants -----------------
            obsT = cst.tile([P, 2 * T], dt.bfloat16, tag="obsT")
            nc.sync.dma_start(
                out=obsT[:].rearrange("p (k t) -> p k t", k=2),
                in_=obsT_e.rearrange("(k p) t -> p k t", p=P),
            )
            w_in = cst.tile([P, 2 * D], dt.bfloat16, tag="w_in")
            nc.sync.dma_start(
                out=w_in[:].rearrange("p (k d) -> p k d", k=2),
                in_=w_in_e.rearrange("(k p) d -> p k d", p=P),
            )
            w_dt = cst.tile([P, KT * H], dt.bfloat16, tag="w_dt")
            nc.sync.dma_start(
                out=w_dt[:].rearrange("p (k h) -> p k h", k=KT),
                in_=w_dt_e.rearrange("(k p) h -> p k h", p=P),
            )
            w_yo = cst.tile([P, KT * U], dt.bfloat16, tag="w_yo")
            nc.scalar.dma_start(
                out=w_yo[:].rearrange("p (k u) -> p k u", k=KT),
                in_=w_yo_e.rearrange("(k p) u -> p k u", p=P),
            )
            w_hd = cst.tile([P, 2 * A], dt.bfloat16, tag="w_hd")
            nc.scalar.dma_start(
                out=w_hd[:].rearrange("p (k a) -> p k a", k=2),
                in_=w_hd_e.rearrange("(k p) a -> p k a", p=P),
            )
            neg_a = cst.tile([H, 1], dt.float32, tag="neg_a")
            nc.sync.dma_start(out=neg_a[:], in_=neg_a_e[:])
            dtb = cst.tile([H, 1], dt.float32, tag="dtb")
            nc.sync.dma_start(out=dtb[:], in_=dtb_e[:])
            if with_b_in:
                b_in = cst.tile([P, KT], dt.float32, tag="b_in")
                nc.sync.dma_start(out=b_in[:], in_=bin_e[:])
            if with_b_yo:
                b_yo = cst.tile([P, 2], dt.float32, tag="b_yo")
                nc.sync.dma_start(out=b_yo[:], in_=byo_e[:])
            if with_b_head:
                b_hd = cst.tile([1, A], dt.bfloat16, tag="b_hd")
                nc.sync.dma_start(out=b_hd[:], in_=bhd_e[:])

            ident_f = cst.tile([P, P], dt.float32, tag="ident_f")
            make_identity(nc, ident_f[:])
            ident_pb = cst.tile([P, P], dt.bfloat16, tag="ident_pb")
            make_identity(nc, ident_pb[:])
            # causal ones: UT[j,i] = 1 where j<=i (cumsum matmul)
            ut_ones = cst.tile([L, L], dt.float32, tag="ut_ones")
            nc.gpsimd.memset(ut_ones[:], 1.0)
            nc.gpsimd.affine_select(
                out=ut_ones[:],
                in_=ut_ones[:],
                compare_op=OP.is_ge,
                fill=0.0,
                base=0,
                pattern=[[1, L]],
                channel_multiplier=-1,
            )
            ones_row = cst.tile([1, P], dt.float32, tag="ones_row")
            nc.gpsimd.memset(ones_row[:], 1.0)
            ones2k = cst.tile([1, 4 * T], dt.bfloat16, tag="ones2k")
            nc.gpsimd.memset(ones2k[:], 1.0)

            # ---------------- activations / state ------------------
            xT = cst.tile([P, KT * T], dt.bfloat16, tag="xT")  # (d, t); later = y
            x = cst.tile([P, NCH * D], dt.bfloat16, tag="x")  # (t, d) per t-tile
            # B/C/Cw repacked: 64 partitions, head h at columns [h*T,(h+1)*T)
            bm = cst.tile([N, H * T], dt.bfloat16, tag="bm")
            cm = cst.tile([N, H * T], dt.bfloat16, tag="cm")
            cw = cst.tile([N, H * T], dt.bfloat16, tag="cw")
            y = xT  # reuse: xT fully consumed before first scan Y evac
            zT = obsT  # reuse: obs consumed by phase 1
            logit = cst.tile([P, NCH * A], dt.float32, tag="logit")
            s_st = [
                cst.tile([N, P], dt.bfloat16, tag=f"s{h}", name=f"s{h}")
                for h in range(H)
            ]
            for h in range(H):
                nc.gpsimd.memset(s_st[h][:], 0.0)

            dtT = cst.tile([H, T], dt.float32, tag="dtT")
            logdtT = cst.tile([H, T], dt.float32, tag="logdtT")
            pcumT = cst.tile([H, T], dt.float32, tag="pcumT")  # Pcum rows (h,t)
            npdT = cst.tile([H, T], dt.float32, tag="npdT")  # logdt - Pcum (h,t)
            # two-float (hi/lo bf16) splits for the K=16 bf16 Diff matmul
            pcumH = cst.tile([H, T], dt.bfloat16, tag="pcumH")
            pcumL = cst.tile([H, T], dt.bfloat16, tag="pcumL")
            npdH = cst.tile([H, T], dt.bfloat16, tag="npdH")
            npdL = cst.tile([H, T], dt.bfloat16, tag="npdL")
            plrow = cst.tile([1, NCH * H], dt.float32, tag="plrow")
            u_all = cst.tile([H, T], dt.bfloat16, tag="u_all")  # exp(Pcum)
            cols = cst.tile([P, NCH * 2 * H], dt.float32, tag="cols")  # [PcumCol|dtCol]
            e2c = cst.tile([P, NCH * H], dt.float32, tag="e2c")  # exp(Plast-Pcum)
            dtotc = cst.tile([P, NCH * H], dt.float32, tag="dtotc")
            ulast = cst.tile([1, NCH * H], dt.float32, tag="ulast")
            # K=16 block-diag Diff operands, all 16 (c,hg) groups staged once.
            # Group (c,hg) holds heads h=4*hi+hg; lh16 col block (hg,c) of 128,
            # rp16 col block (hg,c) of 512 (4 heads x L, diag-block layout).
            lh16 = cst.tile([H, 16 * L], dt.bfloat16, tag="lh16")
            rp16 = cst.tile([H, 16 * 4 * L], dt.bfloat16, tag="rp16")
            nc.gpsimd.memset(lh16[:], 1.0)
            nc.gpsimd.memset(rp16[:], 0.0)
            for hi in range(4):
                for r in (1, 3):
                    dst = rp16[4 * hi + r : 4 * hi + r + 1, :].rearrange(
                        "p (g c i t) -> p g c i t", g=4, c=NCH, i=4
                    )[:, :, :, hi : hi + 1, :]
                    nc.sync.dma_start(
                        out=dst,
                        in_=ones2k[0:1, :].rearrange("p (g c t) -> p g c t", g=4, c=NCH),
                    )

            # ---------------- x^T = relu(W_in^T obs^T) (d,t) --------
            for kt in range(KT):
                ps = ps_proj.tile([P, T], dt.float32, tag="proj")
                for ko in range(2):
                    nc.tensor.matmul(
                        ps[:],
                        w_in[:, ko * D + kt * P : ko * D + (kt + 1) * P],
                        obsT[:, ko * T : (ko + 1) * T],
                        start=(ko == 0),
                        stop=(ko == 1),
                    )
                if with_b_in:
                    nc.scalar.activation(
                        xT[:, kt * T : (kt + 1) * T], ps[:], AF.Relu,
                        bias=b_in[:, kt : kt + 1],
                    )
                else:
                    nc.scalar.activation(xT[:, kt * T : (kt + 1) * T], ps[:], AF.Relu)

            # ---------------- x = xT^T via PE transposes (t,d) ------
            for tt in range(NCH):
                for g4 in range(4):
                    ps = ps_proj.tile([P, 4 * P], dt.bfloat16, tag="proj")
                    for kk in range(4):
                        kt = g4 * 4 + kk
                        nc.tensor.transpose(
                            ps[:, kk * P : (kk + 1) * P],
                            xT[:, kt * T + tt * P : kt * T + (tt + 1) * P],
                            ident_pb[:],
                        )
                    nc.vector.tensor_copy(
                        x[:, tt * D + g4 * 512 : tt * D + (g4 + 1) * 512], ps[:]
                    )

            # ---------------- dt chain ------------------------------
            psd = ps_proj.tile([H, T], dt.float32, tag="proj", name="psd")
            for kt in range(KT):
                nc.tensor.matmul(
                    psd[:],
                    w_dt[:, kt * H : (kt + 1) * H],
                    xT[:, kt * T : (kt + 1) * T],
                    start=(kt == 0),
                    stop=(kt == KT - 1),
                )
            # softplus via ln(1+exp(.)) — Softplus shares no ACT table with
            # Exp/Ln on this compiler; exp/ln/relu/copy live in one table.
            ez = wrk.tile([H, T], dt.float32, tag="ez", bufs=1)
            nc.scalar.activation(ez[:], psd[:], AF.Exp, bias=dtb[:])
            nc.vector.tensor_scalar_add(ez[:], ez[:], 1.0)
            nc.scalar.activation(dtT[:], ez[:], AF.Ln)
            nc.scalar.activation(logdtT[:], dtT[:], AF.Ln)

            for c in range(NCH):
                cb = slice(c * L, (c + 1) * L)
                ldec = wrk.tile([H, L], dt.float32, tag="ldec")
                nc.vector.tensor_scalar_mul(ldec[:], dtT[:, cb], neg_a[:])
                pt = ps_tiny.tile([P, 2 * H], dt.float32, tag="tiny")
                nc.tensor.transpose(pt[:, 0:H], ldec[:], ident_f[0:H, 0:H])
                ldec_c = wrk.tile([P, H], dt.float32, tag="ldec_c")
                nc.vector.tensor_copy(ldec_c[:], pt[:, 0:H])
                pp = ps_tiny.tile([H, L], dt.float32, tag="tiny")
                nc.tensor.matmul(pp[:], ldec_c[:], ut_ones[:], start=True, stop=True)
                nc.vector.tensor_copy(pcumT[:, cb], pp[:])
                nc.vector.tensor_sub(npdT[:, cb], logdtT[:, cb], pcumT[:, cb])
                nc.vector.tensor_copy(pcumH[:, cb], pcumT[:, cb])
                nc.vector.tensor_sub(pcumL[:, cb], pcumT[:, cb], pcumH[:, cb])
                nc.vector.tensor_copy(npdH[:, cb], npdT[:, cb])
                nc.vector.tensor_sub(npdL[:, cb], npdT[:, cb], npdH[:, cb])
                nc.scalar.activation(u_all[:, cb], pcumT[:, cb], AF.Exp)
                pt2 = ps_tiny.tile([P, 2 * H], dt.float32, tag="tiny")
                nc.tensor.transpose(pt2[:, 0:H], pcumT[:, cb], ident_f[0:H, 0:H])
                nc.tensor.transpose(pt2[:, H : 2 * H], dtT[:, cb], ident_f[0:H, 0:H])
                co = c * 2 * H
                nc.vector.tensor_copy(cols[:, co : co + 2 * H], pt2[:])
                # PcumLast per head at base partition 0 (row 127 of PcumCol)
                nc.sync.dma_start(
                    out=plrow[:, c * H : (c + 1) * H],
                    in_=cols[L - 1 : L, co : co + H],
                )
                plast = plrow[:, c * H : (c + 1) * H]
                nc.scalar.activation(ulast[:, c * H : (c + 1) * H], plast, AF.Exp)
                pdt = ps_tiny.tile([P, H], dt.float32, tag="tiny")
                nc.tensor.matmul(
                    pdt[:],
                    ones_row[0:1, 0:P],
                    ulast[:, c * H : (c + 1) * H],
                    start=True,
                    stop=True,
                )
                nc.vector.tensor_copy(dtotc[:, c * H : (c + 1) * H], pdt[:])
                ppl = ps_tiny.tile([P, H], dt.float32, tag="tiny")
                nc.tensor.matmul(
                    ppl[:], ones_row[0:1, 0:P], plast, start=True, stop=True
                )
                e2a = wrk.tile([P, H], dt.float32, tag="e2a")
                nc.vector.tensor_sub(e2a[:], ppl[:], cols[:, co : co + H])
                nc.scalar.activation(e2c[:, c * H : (c + 1) * H], e2a[:], AF.Exp)

            # bulk-stage the Diff operand variable rows (all chunks at once):
            # rp16 rows 4hi+0/2 <- pcumH/L of heads {4hi..4hi+3}; lh16 rows
            # 4hi+1/3 <- npdH/L.  dst iter (g=hg, c, t) == src iter (h, c, t).
            for hi in range(4):
                for r, src in ((0, pcumH), (2, pcumL)):
                    dst = rp16[4 * hi + r : 4 * hi + r + 1, :].rearrange(
                        "p (g c i t) -> p g c i t", g=4, c=NCH, i=4
                    )[:, :, :, hi : hi + 1, :]
                    nc.scalar.dma_start(
                        out=dst,
                        in_=src[4 * hi : 4 * hi + 4, :].rearrange(
                            "h (c t) -> h c t", c=NCH
                        ),
                    )
                for r, src in ((1, npdH), (3, npdL)):
                    dst = lh16[4 * hi + r : 4 * hi + r + 1, :].rearrange(
                        "p (g c j) -> p g c j", g=4, c=NCH
                    )
                    nc.scalar.dma_start(
                        out=dst,
                        in_=src[4 * hi : 4 * hi + 4, :].rearrange(
                            "h (c j) -> h c j", c=NCH
                        ),
                    )

            # ---------------- B / C projections ---------------------
            # psum (128, T) holds heads (2mt, 2mt+1); evacuate to bf16 staging
            # then DMA each 64-partition half into the base-0 packed tensors.
            wbv = w_b_e.rearrange("(kt p) m -> p kt m", p=P)
            wcv = w_c_e.rearrange("(kt p) m -> p kt m", p=P)
            for mt in range(8):
                he, ho = 2 * mt, 2 * mt + 1
                wbuf = wrk.tile([P, KT * P], dt.bfloat16, tag="wbs", bufs=3)
                nc.sync.dma_start(
                    out=wbuf[:].rearrange("p (kt m) -> p kt m", kt=KT),
                    in_=wbv[:, :, mt * P : (mt + 1) * P],
                )
                psb = ps_proj.tile([P, T], dt.float32, tag="proj")
                for kt in range(KT):
                    nc.tensor.matmul(
                        psb[:],
                        wbuf[:, kt * P : (kt + 1) * P],
                        xT[:, kt * T : (kt + 1) * T],
                        start=(kt == 0),
                        stop=(kt == KT - 1),
                    )
                btmp = wrk.tile([P, T], dt.bfloat16, tag="btmp", bufs=3)
                nc.vector.tensor_copy(btmp[:], psb[:])
                nc.sync.dma_start(out=bm[:, he * T : (he + 1) * T], in_=btmp[0:N, :])
                nc.sync.dma_start(out=bm[:, ho * T : (ho + 1) * T], in_=btmp[N:P, :])

                wcuf = wrk.tile([P, KT * P], dt.bfloat16, tag="wcs", bufs=3)
                nc.scalar.dma_start(
                    out=wcuf[:].rearrange("p (kt m) -> p kt m", kt=KT),
                    in_=wcv[:, :, mt * P : (mt + 1) * P],
                )
                psc = ps_proj.tile([P, T], dt.float32, tag="proj")
                for kt in range(KT):
                    nc.tensor.matmul(
                        psc[:],
                        wcuf[:, kt * P : (kt + 1) * P],
                        xT[:, kt * T : (kt + 1) * T],
                        start=(kt == 0),
                        stop=(kt == KT - 1),
                    )
                ctmp = wrk.tile([P, T], dt.bfloat16, tag="ctmp", bufs=3)
                nc.vector.tensor_copy(ctmp[:], psc[:])
                nc.sync.dma_start(out=cm[:, he * T : (he + 1) * T], in_=ctmp[0:N, :])
                nc.sync.dma_start(out=cm[:, ho * T : (ho + 1) * T], in_=ctmp[N:P, :])

                # u-scaled C: broadcast u rows across partitions via K=1 matmuls
                urow = wrk.tile([1, 2 * T], dt.bfloat16, tag="urow")
                nc.scalar.dma_start(out=urow[:, 0:T], in_=u_all[he : he + 1, :])
                nc.scalar.dma_start(out=urow[:, T : 2 * T], in_=u_all[ho : ho + 1, :])
                ubp = ps_diff.tile([P, 4 * L], dt.float32, tag="diff", name="ubp")
                nc.tensor.matmul(
                    ubp[0:N, :], ones2k[0:1, 0:N], urow[:, 0:T],
                    start=True, stop=True,
                )
                nc.tensor.matmul(
                    ubp[N:P, :], ones2k[0:1, 0:N], urow[:, T : 2 * T],
                    start=True, stop=True,
                )
                ubc = wrk.tile([P, T], dt.bfloat16, tag="ubc")
                nc.scalar.activation(ubc[:], ubp[:], AF.Copy)
                wtmp = wrk.tile([P, T], dt.bfloat16, tag="wtmp", bufs=3)
                nc.vector.tensor_mul(wtmp[:], psc[:], ubc[:])
                nc.scalar.dma_start(out=cw[:, he * T : (he + 1) * T], in_=wtmp[0:N, :])
                nc.scalar.dma_start(out=cw[:, ho * T : (ho + 1) * T], in_=wtmp[N:P, :])

            # ---------------- scan ----------------------------------
            # y cols (h, t) with h = 4*hi + hg: strided evac view per group.
            yv4 = y[:].rearrange("p (i g t) -> p i g t", i=4, g=4)
            # e_sb / gw / bd rings carved from the dead w_in tile (bf16,
            # (P, 4096) = 8 x 512-col slots).  w_in's last read is phase 1.
            esb_ring = [w_in[:, i * 512 : (i + 1) * 512] for i in range(3)]
            gw_ring = [w_in[:, (3 + i) * 512 : (4 + i) * 512] for i in range(3)]
            bd_ring = [
                w_in[:, 3072 + i * 256 : 3072 + (i + 1) * 256] for i in range(2)
            ]
            # chunk-major: consecutive groups touch different heads, so the
            # serial per-head state chain is 4 groups apart and the scan
            # pipelines without waiting on the S-update chain.  Emission is
            # software-pipelined: group g's Diff+G matmuls go to the PE before
            # group g-1's Y/S block, hiding the exp->mask->mul latency.
            pend = None
            gidx = 0
            for c in range(NCH):
                for hg in range(4):
                    grp = hg * 4 + c
                    dbank = ps_diff.tile([P, 4 * L], dt.float32, tag="diff")
                    nc.tensor.matmul(
                        dbank[:],
                        lh16[:, grp * L : (grp + 1) * L],
                        rp16[:, grp * 4 * L : (grp + 1) * 4 * L],
                        start=True,
                        stop=True,
                    )
                    gbank = ps_gy.tile([P, 4 * L], dt.float32, tag="gy")
                    for hi in range(4):
                        h = 4 * hi + hg
                        hb = slice(h * T + c * L, h * T + (c + 1) * L)
                        nc.tensor.matmul(
                            gbank[:, hi * L : (hi + 1) * L],
                            bm[:, hb],
                            cm[:, hb],
                            start=True,
                            stop=True,
                        )
                    e_sb = esb_ring[gidx % 3]
                    nc.scalar.activation(e_sb, dbank[:], AF.Exp)
                    # causal mask: keep i>=j else 0 (kills the exp-overflow infs)
                    nc.gpsimd.affine_select(
                        out=e_sb,
                        in_=e_sb,
                        compare_op=OP.is_ge,
                        fill=0.0,
                        base=0,
                        pattern=[[0, 4], [1, L]],
                        channel_multiplier=-1,
                    )
                    gw = gw_ring[gidx % 3]
                    nc.vector.tensor_mul(gw, gbank[:], e_sb)
                    if pend is not None:
                        pend()

                    def consume(c=c, hg=hg, gw=gw, gi=gidx):
                        ybank = ps_gy.tile([P, 4 * L], dt.float32, tag="gy")
                        btr = ps_tiny.tile([P, 4 * N], dt.bfloat16, tag="tiny")
                        sdb = ps_sd.tile([N, 4 * P], dt.float32, tag="sd")
                        bd = bd_ring[gi % 2]
                        for hi in range(4):
                            h = 4 * hi + hg
                            hb = slice(h * T + c * L, h * T + (c + 1) * L)
                            xc = x[:, c * D + h * P : c * D + (h + 1) * P]
                            nc.tensor.matmul(
                                ybank[:, hi * L : (hi + 1) * L],
                                xc,
                                gw[:, hi * L : (hi + 1) * L],
                                start=True,
                                stop=False,
                            )
                            nc.tensor.matmul(
                                ybank[:, hi * L : (hi + 1) * L],
                                s_st[h][:],
                                cw[:, hb],
                                start=False,
                                stop=True,
                            )
                            nc.tensor.transpose(
                                btr[:, hi * N : (hi + 1) * N],
                                bm[:, hb],
                                ident_pb[0:N, 0:N],
                            )
                            nc.vector.tensor_scalar(
                                bd[:, hi * N : (hi + 1) * N],
                                btr[:, hi * N : (hi + 1) * N],
                                e2c[:, c * H + h : c * H + h + 1],
                                cols[:, c * 2 * H + H + h : c * 2 * H + H + h + 1],
                                op0=OP.mult,
                                op1=OP.mult,
                            )
                            sds = sdb[:, hi * P : (hi + 1) * P]
                            nc.tensor.matmul(
                                sds, bd[:, hi * N : (hi + 1) * N], xc,
                                start=True, stop=True,
                            )
                            nc.vector.scalar_tensor_tensor(
                                s_st[h][:],
                                s_st[h][:],
                                dtotc[0:N, c * H + h : c * H + h + 1],
                                sds,
                                op0=OP.mult,
                                op1=OP.add,
                            )
                        # Y evac: psum (p, (hi,L)) -> y cols (4*hi+hg, c*L..)
                        nc.scalar.activation(
                            yv4[:, :, hg : hg + 1, c * L : (c + 1) * L],
                            ybank[:].rearrange("p (i t) -> p i t", i=4),
                            AF.Copy,
                        )
                        # z accumulation: after chunk 3, head h's y is final.
                        if c == NCH - 1:
                            for ut in range(2):
                                zp = z_ps[ut]
                                for hi in range(4):
                                    h = 4 * hi + hg
                                    nc.tensor.matmul(
                                        zp[:],
                                        w_yo[:, h * U + ut * P : h * U + (ut + 1) * P],
                                        y[:, h * T : (h + 1) * T],
                                        start=(hg == 0 and hi == 0),
                                        stop=(hg == 3 and hi == 3),
                                    )

                    pend = consume
                    gidx += 1
                if c == NCH - 2:
                    # acquire the z psum banks before the last chunk's groups
                    z_ps = [
                        ps_proj.tile([P, T], dt.float32, tag="proj", name=f"z{ut}")
                        for ut in range(2)
                    ]
            pend()

            # ---------------- z = relu(y W_yo) (u,t) ----------------
            for ut in range(2):
                if with_b_yo:
                    nc.scalar.activation(
                        zT[:, ut * T : (ut + 1) * T], z_ps[ut][:], AF.Relu,
                        bias=b_yo[:, ut : ut + 1],
                    )
                else:
                    nc.scalar.activation(
                        zT[:, ut * T : (ut + 1) * T], z_ps[ut][:], AF.Relu
                    )

            # ---------------- logits --------------------------------
            for tt in range(NCH):
                ps = ps_proj.tile([P, A], dt.float32, tag="proj")
                nmm = 3 if with_b_head else 2
                for ut in range(2):
                    nc.tensor.matmul(
                        ps[:],
                        zT[:, ut * T + tt * P : ut * T + (tt + 1) * P],
                        w_hd[:, ut * A : (ut + 1) * A],
                        start=(ut == 0),
                        stop=(ut == nmm - 1),
                    )
                if with_b_head:
                    nc.tensor.matmul(
                        ps[:],
                        ones2k[0:1, tt * P : (tt + 1) * P],
                        b_hd[:],
                        start=False,
                        stop=True,
                    )
                nc.scalar.activation(logit[:, tt * A : (tt + 1) * A], ps[:], AF.Copy)
                nc.sync.dma_start(
                    out=out_e[tt * P : (tt + 1) * P, :],
                    in_=logit[:, tt * A : (tt + 1) * A],
                )

    _split_multi_waits(nc)
    return nc


def kernel(obs, W_in, b_in, A_log, dt_bias, W_dt, W_B, W_C, W_yo, b_yo, W_head, b_head):
    _inject_axon_hooks()
    _patch_tile()
    from concourse.bass_utils import run_bass_kernel_spmd

    obs = np.asarray(obs, dtype=np.float32)
    flags = (
        bool(np.any(np.asarray(b_in) != 0)),
        bool(np.any(np.asarray(b_yo) != 0)),
        bool(np.any(np.asarray(b_head) != 0)),
    )
    # First call: build once (the verified path). Repeat calls in one
    # process rebuild a fresh graph — re-executing a previously-run nc with
    # new inputs has crashed the exec unit (NRT status 101) in testing.
    if flags not in _CACHE:
        _CACHE[flags] = _build(*flags)
    elif _EXECUTED.get(flags):
        _CACHE[flags] = _build(*flags)
    nc = _CACHE[flags]
    _EXECUTED[flags] = True

    obsT = obs.reshape(T, BSZ, OBSD).transpose(1, 2, 0)  # (B, 256, T)
    base = {
        "w_in": np.ascontiguousarray(W_in).astype(BF16),
        "w_dt": np.ascontiguousarray(W_dt).astype(BF16),
        "w_b": np.ascontiguousarray(W_B).astype(BF16),
        "w_c": np.ascontiguousarray(W_C).astype(BF16),
        "w_yo": np.ascontiguousarray(W_yo).astype(BF16),
        "w_hd": np.ascontiguousarray(W_head).astype(BF16),
        "neg_a": (-np.exp(np.asarray(A_log, np.float64)))
        .astype(np.float32)
        .reshape(H, 1),
        "dtb": np.asarray(dt_bias, np.float32).reshape(H, 1),
    }
    if flags[0]:
        base["b_in"] = np.ascontiguousarray(
            np.asarray(b_in, np.float32).reshape(KT, P).T
        )
    if flags[1]:
        base["b_yo"] = np.ascontiguousarray(
            np.asarray(b_yo, np.float32).reshape(2, P).T
        )
    if flags[2]:
        base["b_hd"] = np.asarray(b_head).astype(BF16).reshape(1, A)
    in_maps = [
        dict(base, obsT=np.ascontiguousarray(obsT[c]).astype(BF16)) for c in range(BSZ)
    ]
    global _last_in_maps
    _last_in_maps = in_maps
    res = run_bass_kernel_spmd(nc, in_maps, core_ids=list(range(BSZ)))
    out = np.stack([res.results[c]["out"] for c in range(BSZ)], axis=1)
    return out.astype(np.float32)
